# revision 28
# baseline (speedup 1.0000x reference)
"""MaxPool3d (kernel=3, stride=2, padding=1) on Trainium2, 8 NeuronCores.

Input  x: (2, 32, 128, 128, 128) f32  ->  Output: (2, 32, 64, 64, 64) f32.

Sharding: 64 (b, c) slices data-parallel; each core gets 8 slices as 4
slice-pairs (a pair packs 2 slices into the 128 SBUF partitions: partition
64*s + d/2, parity slabs for even/odd d).

Design (measured 205us vs 230-238us baseline):
  - DVE is the end-to-end bottleneck (~28K output elements/partition/chunk
    across H, D, W pools regardless of op order, at ~2.3 elem/ns dense plus
    a fixed issue/drain overhead per op when saturated). So: MINIMIZE DVE
    OP COUNT. Both d-parities live in ONE x tile [128, 2, hc, W] (two DMA
    writers), so each H op covers both parities at once: 7 DVE ops per
    chunk (H1 H2 bnd D1 D2 W1 W2) instead of 11.
  - Pool order H -> D -> W; the partition shift (O[od-1] term of the D
    pool) runs on the idle TensorE (one-hot shift matrix, one bank-sized
    matmul per 512 f32 into double-buffered PSUM halves) and ACT copies
    PSUM -> SBUF bf16, so DVE's D2 stays one dense 2x op. This moves the
    shift off the DMA engines (the other near-saturated resource).
  - Output stored as bf16 (halves store DMA-engine time); host casts to
    f32. bf16 compute already bounds rel err at ~2^-8 << 2e-2 gate.
  - Store triggers issue from the idle SP (sync) engine so their tile
    waits never block the ACT copies that feed DVE.
  - Small first/last chunks shrink the pipeline ramp and tail.

Window math (PADDING=1): out[o] = max(in[2o-1], in[2o], in[2o+1]).
D axis: out[od] = max(E[od], O[od], O[od-1]); O[od-1] via TensorE shift;
partitions 0/64 of the shift duplicate rows 0/64 (idempotent under max).
"""

import sys

sys.path.insert(0, "/opt/trn_rl_repo")

import numpy as np

B, C, D, H, W = 2, 32, 128, 128, 128
OD, OH, OW = 64, 64, 64
N_CORES = 8
SLICES_PER_CORE = (B * C) // N_CORES  # 8
PAIRS = SLICES_PER_CORE // 2  # 4
HCMAX = 56
# per-pair chunk schedules: small first chunks start DVE early (ramp);
# small last chunk shrinks the after-last-load tail
CHUNKS = [
    [16, 56, 56],  # pair 0: ramp
    [56, 56, 16],
    [56, 56, 16],
    [56, 56, 16],  # pair 3: tail
]
assert all(sum(cs) == H and max(cs) <= HCMAX for cs in CHUNKS)

_cache = {}
INST_LABELS = {}


def _lab(inst, label):
    INST_LABELS[inst.ins.name] = label
    return inst


def _shift_matrix():
    """S[k, p] = 1 iff k == src(p): out[p] = in[p-1] within each 64-slab,
    rows 0/64 duplicating (idempotent under max)."""
    import ml_dtypes

    S = np.zeros((128, 128), dtype=ml_dtypes.bfloat16)
    for p in range(128):
        q = p % 64
        src = p - 1 if q > 0 else p
        S[src, p] = 1
    return S


def _build():
    import concourse.mybir as mybir
    from concourse import bacc
    from concourse.tile import TileContext

    f32 = mybir.dt.float32
    bf16 = mybir.dt.bfloat16
    Copy = mybir.ActivationFunctionType.Copy
    nc = bacc.Bacc()
    x_ext = nc.declare_dram_parameter(
        "x_shard", [SLICES_PER_CORE, D, H, W], f32, isOutput=False
    )
    w_ext = nc.declare_dram_parameter("shift_w", [128, 128], bf16, isOutput=False)
    y_ext = nc.declare_dram_parameter(
        "y_shard", [SLICES_PER_CORE, OD, OH, OW], bf16, isOutput=True
    )

    OHCMAX = HCMAX // 2
    OHH = OHCMAX // 2  # half-chunk rows for the PSUM double buffer

    with TileContext(nc) as tc:
        with (
            tc.tile_pool(name="wpool", bufs=1) as wpool,
            tc.tile_pool(name="xpool", bufs=2) as xpool,
            tc.tile_pool(name="hpool", bufs=2) as hpool,
            tc.tile_pool(name="spool", bufs=1) as spool,
            tc.tile_pool(name="gpool", bufs=1) as gpool,
            tc.tile_pool(name="ypool", bufs=2) as ypool,
            tc.tile_pool(name="cpool", bufs=2) as cpool,
            tc.psum_pool(name="mpool", bufs=2) as mpool,
        ):
            Sw = wpool.tile([128, 128], bf16, name="Sw", tag="Sw")
            _lab(nc.sync.dma_start(out=Sw[:, :], in_=w_ext[:, :]), "ldS")

            for p in range(PAIRS):
                s0 = 2 * p
                xprev = None
                h0 = 0
                last = len(CHUNKS[p]) - 1
                for c, hc in enumerate(CHUNKS[p]):
                    ohc = hc // 2
                    oh0 = h0 // 2
                    # both parities in one tile: free dims [par, h, w]
                    xt = xpool.tile(
                        [128, 2, HCMAX, W], bf16, name="xt", tag="xt"
                    )
                    # cast loads f32 -> bf16 (SWDGE); O parity first so the
                    # PE shift's input is ready earlier
                    for par in (1, 0):
                        nm = "E" if par == 0 else "O"
                        _lab(nc.gpsimd.dma_start(
                            out=xt[:, par, 0:hc, :],
                            in_=x_ext[s0 : s0 + 2, par : D : 2, h0 : h0 + hc, :],
                        ), f"p{p}c{c}ld{nm}")
                    # ---- H pool, both parities per op (unit-stride, 2x) ----
                    Hb = hpool.tile(
                        [128, 2, OHCMAX, W], bf16, name="Hb", tag="Hb"
                    )
                    _lab(nc.vector.tensor_max(
                        out=Hb[:, :, 0:ohc, :],
                        in0=xt[:, :, 0:hc:2, :],
                        in1=xt[:, :, 1:hc:2, :],
                    ), f"p{p}c{c}H1")
                    _lab(nc.vector.tensor_max(
                        out=Hb[:, :, 1:ohc, :],
                        in0=Hb[:, :, 1:ohc, :],
                        in1=xt[:, :, 1 : hc - 2 : 2, :],
                    ), f"p{p}c{c}H2")
                    if c > 0:
                        # boundary: h = 2*oh0 - 1 = prev chunk's last row
                        _lab(nc.vector.tensor_max(
                            out=Hb[:, :, 0:1, :],
                            in0=Hb[:, :, 0:1, :],
                            in1=xprev[:, :, 0:1, :],
                        ), f"p{p}c{c}bnd")
                    # carry row out so xt's lifetime ends in this chunk;
                    # a tiny DVE copy (~0.3us) rather than ACT: its load-
                    # wait on ACT blocked the PSUM->SBUF copies that feed
                    # D2 whenever the scheduler hoisted it across chunks
                    if c < last:
                        cw = cpool.tile([128, 2, 1, W], bf16, name="cw", tag="cw")
                        _lab(nc.vector.tensor_copy(
                            out=cw[:, :, 0:1, :],
                            in_=xt[:, :, hc - 1 : hc, :],
                        ), f"p{p}c{c}cw")
                        xprev = cw
                    # ---- partition shift of the O-parity H rows on TensorE,
                    # two PSUM halves; ACT copies into one Gs tile ----
                    Gs = spool.tile([128, OHCMAX, W], bf16, name="Gs", tag="Gs")
                    for hb, r0 in enumerate(range(0, ohc, OHH)):
                        r1 = min(r0 + OHH, ohc)
                        mm = mpool.tile([128, OHH, W], f32, name="mm", tag="mm")
                        # one matmul per PSUM bank (512 f32 = 4 rows)
                        for mb in range(r0, r1, 4):
                            me = min(mb + 4, r1)
                            _lab(nc.tensor.matmul(
                                out=mm[:, mb - r0 : me - r0, :],
                                lhsT=Sw,
                                rhs=Hb[:, 1, mb:me, :],
                                start=True,
                                stop=True,
                            ), f"p{p}c{c}mm{mb}")
                        _lab(nc.scalar.activation(
                            out=Gs[:, r0:r1, :],
                            in_=mm[:, 0 : r1 - r0, :],
                            func=Copy,
                        ), f"p{p}c{c}cp{hb}")
                    # ---- D pool (dense bf16, 2x) ----
                    Gd = gpool.tile([128, OHCMAX, W], bf16, name="Gd", tag="Gd")
                    _lab(nc.vector.tensor_max(
                        out=Gd[:, 0:ohc, :],
                        in0=Hb[:, 0, 0:ohc, :],
                        in1=Hb[:, 1, 0:ohc, :],
                    ), f"p{p}c{c}D1")
                    _lab(nc.vector.tensor_max(
                        out=Gd[:, 0:ohc, :],
                        in0=Gd[:, 0:ohc, :],
                        in1=Gs[:, 0:ohc, :],
                    ), f"p{p}c{c}D2")
                    # ---- W pool (strided, 1x), once per chunk ----
                    Yt = ypool.tile([128, OHCMAX, OW], bf16, name="Yt", tag="Yt")
                    _lab(nc.vector.tensor_max(
                        out=Yt[:, 0:ohc, 0:OW],
                        in0=Gd[:, 0:ohc, 0:W:2],
                        in1=Gd[:, 0:ohc, 1:W:2],
                    ), f"p{p}c{c}W1")
                    _lab(nc.vector.tensor_max(
                        out=Yt[:, 0:ohc, 1:OW],
                        in0=Yt[:, 0:ohc, 1:OW],
                        in1=Gd[:, 0:ohc, 1 : W - 2 : 2],
                    ), f"p{p}c{c}W2")
                    _lab(nc.sync.dma_start(
                        out=y_ext[s0 : s0 + 2, :, oh0 : oh0 + ohc, :],
                        in_=Yt[:, 0:ohc, :],
                    ), f"p{p}c{c}st")
                    h0 += hc
    nc.compile()
    return nc


def _get_nc():
    if "nc" not in _cache:
        _cache["nc"] = _build()
    return _cache["nc"]


def run(x: np.ndarray, **spmd_kwargs):
    """Run the SPMD kernel; returns the BassKernelResults (for tracing)."""
    from concourse.bass_utils import run_bass_kernel_spmd

    nc = _get_nc()
    xs = np.ascontiguousarray(x, dtype=np.float32).reshape(B * C, D, H, W)
    Sw = _shift_matrix()
    in_maps = [
        {
            "x_shard": np.ascontiguousarray(
                xs[SLICES_PER_CORE * i : SLICES_PER_CORE * (i + 1)]
            ),
            "shift_w": Sw,
        }
        for i in range(N_CORES)
    ]
    return run_bass_kernel_spmd(nc, in_maps, list(range(N_CORES)), **spmd_kwargs)


def kernel(x: np.ndarray) -> np.ndarray:
    res = run(x)
    out = np.stack(
        [np.asarray(res.results[i]["y_shard"]) for i in range(N_CORES)]
    ).astype(np.float32)
    return out.reshape(B, C, OD, OH, OW)


# revision 29
# speedup vs baseline: 1.3015x; 1.3015x over previous
"""MaxPool3d (kernel=3, stride=2, padding=1) on Trainium2, 8 NeuronCores.

Input  x: (2, 32, 128, 128, 128) f32  ->  Output: (2, 32, 64, 64, 64) f32.

Sharding: 64 (b, c) slices data-parallel; each core gets 8 slices as 4
slice-pairs (a pair packs 2 slices into the 128 SBUF partitions: partition
64*s + d/2, parity slabs for even/odd d).

Design (measured 205us vs 230-238us baseline):
  - DVE is the end-to-end bottleneck (~28K output elements/partition/chunk
    across H, D, W pools regardless of op order, at ~2.3 elem/ns dense plus
    a fixed issue/drain overhead per op when saturated). So: MINIMIZE DVE
    OP COUNT. Both d-parities live in ONE x tile [128, 2, hc, W] (two DMA
    writers), so each H op covers both parities at once: 7 DVE ops per
    chunk (H1 H2 bnd D1 D2 W1 W2) instead of 11.
  - Pool order H -> D -> W; the partition shift (O[od-1] term of the D
    pool) runs on the idle TensorE (one-hot shift matrix, one bank-sized
    matmul per 512 f32 into double-buffered PSUM halves) and ACT copies
    PSUM -> SBUF bf16, so DVE's D2 stays one dense 2x op. This moves the
    shift off the DMA engines (the other near-saturated resource).
  - Output stored as bf16 (halves store DMA-engine time); host casts to
    f32. bf16 compute already bounds rel err at ~2^-8 << 2e-2 gate.
  - Store triggers issue from the idle SP (sync) engine so their tile
    waits never block the ACT copies that feed DVE.
  - Small first/last chunks shrink the pipeline ramp and tail.

Window math (PADDING=1): out[o] = max(in[2o-1], in[2o], in[2o+1]).
D axis: out[od] = max(E[od], O[od], O[od-1]); O[od-1] via TensorE shift;
partitions 0/64 of the shift duplicate rows 0/64 (idempotent under max).
"""

import sys

sys.path.insert(0, "/opt/trn_rl_repo")

import numpy as np

B, C, D, H, W = 2, 32, 128, 128, 128
OD, OH, OW = 64, 64, 64
N_CORES = 8
SLICES_PER_CORE = (B * C) // N_CORES  # 8
PAIRS = SLICES_PER_CORE // 2  # 4
HCMAX = 64
# per-pair chunk schedules: small first chunks start DVE early (ramp);
# small last chunk shrinks the after-last-load tail
CHUNKS = [
    [16, 48, 64],  # pair 0: ramp
    [64, 64],
    [64, 64],
    [64, 48, 16],  # pair 3: tail
]
assert all(sum(cs) == H and max(cs) <= HCMAX for cs in CHUNKS)

_cache = {}
INST_LABELS = {}


def _lab(inst, label):
    INST_LABELS[inst.ins.name] = label
    return inst


def _shift_matrix():
    """S[k, p] = 1 iff k == src(p): out[p] = in[p-1] within each 64-slab,
    rows 0/64 duplicating (idempotent under max)."""
    import ml_dtypes

    S = np.zeros((128, 128), dtype=ml_dtypes.bfloat16)
    for p in range(128):
        q = p % 64
        src = p - 1 if q > 0 else p
        S[src, p] = 1
    return S


def _build():
    import concourse.mybir as mybir
    from concourse import bacc
    from concourse.tile import TileContext

    f32 = mybir.dt.float32
    bf16 = mybir.dt.bfloat16
    Copy = mybir.ActivationFunctionType.Copy
    nc = bacc.Bacc()
    x_ext = nc.declare_dram_parameter(
        "x_shard", [SLICES_PER_CORE, D, H, W], f32, isOutput=False
    )
    w_ext = nc.declare_dram_parameter("shift_w", [128, 128], bf16, isOutput=False)
    y_ext = nc.declare_dram_parameter(
        "y_shard", [SLICES_PER_CORE, OD, OH, OW], bf16, isOutput=True
    )

    OHCMAX = HCMAX // 2
    OHH = OHCMAX // 2  # half-chunk rows for the PSUM double buffer

    with TileContext(nc) as tc:
        with (
            tc.tile_pool(name="wpool", bufs=1) as wpool,
            tc.tile_pool(name="xpool", bufs=2) as xpool,
            tc.tile_pool(name="hpool", bufs=1) as hpool,
            tc.tile_pool(name="spool", bufs=1) as spool,
            tc.tile_pool(name="gpool", bufs=1) as gpool,
            tc.tile_pool(name="ypool", bufs=2) as ypool,
            tc.tile_pool(name="cpool", bufs=2) as cpool,
            tc.psum_pool(name="mpool", bufs=2) as mpool,
        ):
            Sw = wpool.tile([128, 128], bf16, name="Sw", tag="Sw")
            _lab(nc.sync.dma_start(out=Sw[:, :], in_=w_ext[:, :]), "ldS")

            for p in range(PAIRS):
                s0 = 2 * p
                xprev = None
                h0 = 0
                last = len(CHUNKS[p]) - 1
                for c, hc in enumerate(CHUNKS[p]):
                    ohc = hc // 2
                    oh0 = h0 // 2
                    # both parities in one tile: free dims [par, h, w]
                    xt = xpool.tile(
                        [128, 2, HCMAX, W], bf16, name="xt", tag="xt"
                    )
                    # cast loads f32 -> bf16 (SWDGE); O parity first so the
                    # PE shift's input is ready earlier
                    for par in (1, 0):
                        nm = "E" if par == 0 else "O"
                        _lab(nc.gpsimd.dma_start(
                            out=xt[:, par, 0:hc, :],
                            in_=x_ext[s0 : s0 + 2, par : D : 2, h0 : h0 + hc, :],
                        ), f"p{p}c{c}ld{nm}")
                    # ---- H pool, both parities per op (unit-stride, 2x) ----
                    Hb = hpool.tile(
                        [128, 2, OHCMAX, W], bf16, name="Hb", tag="Hb"
                    )
                    _lab(nc.vector.tensor_max(
                        out=Hb[:, :, 0:ohc, :],
                        in0=xt[:, :, 0:hc:2, :],
                        in1=xt[:, :, 1:hc:2, :],
                    ), f"p{p}c{c}H1")
                    _lab(nc.vector.tensor_max(
                        out=Hb[:, :, 1:ohc, :],
                        in0=Hb[:, :, 1:ohc, :],
                        in1=xt[:, :, 1 : hc - 2 : 2, :],
                    ), f"p{p}c{c}H2")
                    if c > 0:
                        # boundary: h = 2*oh0 - 1 = prev chunk's last row
                        _lab(nc.vector.tensor_max(
                            out=Hb[:, :, 0:1, :],
                            in0=Hb[:, :, 0:1, :],
                            in1=xprev[:, :, 0:1, :],
                        ), f"p{p}c{c}bnd")
                    # carry row out so xt's lifetime ends in this chunk
                    if c < last:
                        cw = cpool.tile([128, 2, 1, W], bf16, name="cw", tag="cw")
                        _lab(nc.scalar.activation(
                            out=cw[:, :, 0:1, :],
                            in_=xt[:, :, hc - 1 : hc, :],
                            func=Copy,
                        ), f"p{p}c{c}cw")
                        xprev = cw
                    # ---- partition shift of the O-parity H rows on TensorE,
                    # two PSUM halves; ACT copies into one Gs tile ----
                    Gs = spool.tile([128, OHCMAX, W], bf16, name="Gs", tag="Gs")
                    for hb, r0 in enumerate(range(0, ohc, OHH)):
                        r1 = min(r0 + OHH, ohc)
                        mm = mpool.tile([128, OHH, W], f32, name="mm", tag="mm")
                        # one matmul per PSUM bank (512 f32 = 4 rows)
                        for mb in range(r0, r1, 4):
                            me = min(mb + 4, r1)
                            _lab(nc.tensor.matmul(
                                out=mm[:, mb - r0 : me - r0, :],
                                lhsT=Sw,
                                rhs=Hb[:, 1, mb:me, :],
                                start=True,
                                stop=True,
                            ), f"p{p}c{c}mm{mb}")
                        _lab(nc.scalar.activation(
                            out=Gs[:, r0:r1, :],
                            in_=mm[:, 0 : r1 - r0, :],
                            func=Copy,
                        ), f"p{p}c{c}cp{hb}")
                    # ---- D pool (dense bf16, 2x) ----
                    Gd = gpool.tile([128, OHCMAX, W], bf16, name="Gd", tag="Gd")
                    _lab(nc.vector.tensor_max(
                        out=Gd[:, 0:ohc, :],
                        in0=Hb[:, 0, 0:ohc, :],
                        in1=Hb[:, 1, 0:ohc, :],
                    ), f"p{p}c{c}D1")
                    _lab(nc.vector.tensor_max(
                        out=Gd[:, 0:ohc, :],
                        in0=Gd[:, 0:ohc, :],
                        in1=Gs[:, 0:ohc, :],
                    ), f"p{p}c{c}D2")
                    # ---- W pool (strided, 1x), once per chunk ----
                    Yt = ypool.tile([128, OHCMAX, OW], bf16, name="Yt", tag="Yt")
                    _lab(nc.vector.tensor_max(
                        out=Yt[:, 0:ohc, 0:OW],
                        in0=Gd[:, 0:ohc, 0:W:2],
                        in1=Gd[:, 0:ohc, 1:W:2],
                    ), f"p{p}c{c}W1")
                    _lab(nc.vector.tensor_max(
                        out=Yt[:, 0:ohc, 1:OW],
                        in0=Yt[:, 0:ohc, 1:OW],
                        in1=Gd[:, 0:ohc, 1 : W - 2 : 2],
                    ), f"p{p}c{c}W2")
                    _lab(nc.sync.dma_start(
                        out=y_ext[s0 : s0 + 2, :, oh0 : oh0 + ohc, :],
                        in_=Yt[:, 0:ohc, :],
                    ), f"p{p}c{c}st")
                    h0 += hc
    nc.compile()
    return nc


def _get_nc():
    if "nc" not in _cache:
        _cache["nc"] = _build()
    return _cache["nc"]


def run(x: np.ndarray, **spmd_kwargs):
    """Run the SPMD kernel; returns the BassKernelResults (for tracing)."""
    from concourse.bass_utils import run_bass_kernel_spmd

    nc = _get_nc()
    xs = np.ascontiguousarray(x, dtype=np.float32).reshape(B * C, D, H, W)
    Sw = _shift_matrix()
    in_maps = [
        {
            "x_shard": np.ascontiguousarray(
                xs[SLICES_PER_CORE * i : SLICES_PER_CORE * (i + 1)]
            ),
            "shift_w": Sw,
        }
        for i in range(N_CORES)
    ]
    return run_bass_kernel_spmd(nc, in_maps, list(range(N_CORES)), **spmd_kwargs)


def kernel(x: np.ndarray) -> np.ndarray:
    res = run(x)
    out = np.stack(
        [np.asarray(res.results[i]["y_shard"]) for i in range(N_CORES)]
    ).astype(np.float32)
    return out.reshape(B, C, OD, OH, OW)
